# revision 1
# baseline (speedup 1.0000x reference)
"""Trainium2 Bass kernel for nn_Attention_41085657153633.

B=8, N=1024, C=384, H=6, D=64 attention with per-head q/k normalization
(mean/std over head_dim, ddof=1), softmax(QK^T/sqrt(D)) @ V, output proj.

Sharding: data-parallel over B — each of the 8 NeuronCores handles one
batch element end-to-end (no collectives).

Per-core dataflow (fp32r matmuls; bf16 for the exp-score/V stage):
  - host supplies x[b]^T [384,1024]; QKV^T computed head-major for Q,K
    ([d on partitions, tokens on free] — the layout QK^T wants) and
    token-major for V (what AV wants as the stationary operand). V is
    stored as [v_h | 64 ones columns] per head so the AV matmul's rows
    64-127 accumulate the softmax denominator already broadcast across
    64 partitions, for free in the M dimension.
  - q/k normalization over the partition (d) axis via block-diagonal
    ones matmuls: mean_bcast = blockdiag(1/64) @ q and ssq_bcast =
    blockdiag(1/64) @ (q-mean)^2 come out broadcast across each head's
    64 partitions by construction; rstd = exp(-0.5*ln(ssq) + b) with the
    ddof=1 correction folded into the ACT bias (Rsqrt table is blocked).
  - scores S^T[s,t] per head via K^T x Q^T (contraction over d=64);
    head pairs occupy PE row groups T0/T8 (64-row mode), strictly
    alternating so each weight load overlaps the other group's matmul.
  - exp on ACT with the 1/sqrt(D) scale folded in (softmax max-sub
    skipped: |S/8| <~ 7, safe in fp32), bf16 scores for AV; 1/denom =
    exp(-ln d) on ACT (ln/exp/softmax-exp share one table set — no
    table switches in the whole kernel).
  - phases are emitted so each pair's ACT-heavy exp stretch overlaps
    the next pair's PE-heavy QKV/norm, PE-mode switches are grouped,
    and a warmup matmul burst opens the HAM clock gate during the
    input DMA wait; output proj from attn_out^T, host transposes.
"""

import sys

sys.path.insert(0, "/opt/trn_rl_repo")

import json

import numpy as np

B, N, C = 8, 1024, 384
H, D = 6, 64
NCORES = 8

_prog = None


def _install_multiwait_fixup():
    """This container's walrus build rejects >1 sync wait per instruction
    ("Too many sync wait commands"). Rewrite the BIR JSON before compile:
    hoist extra waits onto single-wait EventSemaphore instructions
    inserted just before the owner on the same engine (engines dispatch
    in program order, so the gating is preserved)."""
    from concourse import bass2jax, bass_utils

    if getattr(bass_utils, "_multiwait_fixup", False):
        return
    bass_utils._multiwait_fixup = True

    orig = bass_utils.compile_bir_kernel

    def _split(bir_json: bytes) -> bytes:
        j = json.loads(bir_json)
        for fn in j.get("functions", []):
            for bb in fn.get("blocks", []):
                out = []
                for inst in bb.get("instructions", []):
                    si = inst.get("sync_info")
                    waits = si.get("on_wait", []) if si else []
                    if len(waits) > 1:
                        for k, w in enumerate(waits[:-1]):
                            out.append({
                                "debug": inst.get("debug", 0),
                                "engine": inst["engine"],
                                "ins": [],
                                "outs": [],
                                "name": f"{inst['name']}-sw{k}",
                                "opcode": "EventSemaphore",
                                "sync_info": {"on_update": [], "on_wait": [w]},
                            })
                        si["on_wait"] = [waits[-1]]
                    out.append(inst)
                bb["instructions"] = out
        return json.dumps(j).encode()

    def patched(bir_json, tmpdir, neff_name="file.neff"):
        return orig(_split(bir_json), tmpdir, neff_name)

    bass_utils.compile_bir_kernel = patched
    bass2jax.compile_bir_kernel = patched




def _build():
    import concourse.bass as bass
    import concourse.tile as tile
    from concourse import mybir

    _install_multiwait_fixup()

    F32 = mybir.dt.float32
    F32R = mybir.dt.float32r
    BF16 = mybir.dt.bfloat16
    EXP = mybir.ActivationFunctionType.Exp
    LN = mybir.ActivationFunctionType.Ln

    nc = bass.Bass("TRN2")
    xT = nc.dram_tensor("xT", [C, N], F32, kind="ExternalInput")
    qkv_wT = nc.dram_tensor("qkv_wT", [C, 3 * C], F32, kind="ExternalInput")
    proj_wT = nc.dram_tensor("proj_wT", [C, C], F32, kind="ExternalInput")
    pb = nc.dram_tensor("pb", [128, 3], F32, kind="ExternalInput")
    bd_mean = nc.dram_tensor("bd_mean", [128, 128], F32, kind="ExternalInput")
    bd_ssq = nc.dram_tensor("bd_ssq", [128, 128], F32, kind="ExternalInput")
    outT = nc.dram_tensor("outT", [C, N], F32, kind="ExternalOutput")

    KC = C // 128   # 3 contraction chunks of the model dim
    TC = N // 128   # 8 token chunks
    HALF = N // 512  # 2 free-dim halves per 1024-token row

    with tile.TileContext(nc) as tc:
      with nc.allow_low_precision(reason="f32r/bf16 matmul intermediates"):
        with tc.tile_pool(name="consts", bufs=1) as consts, \
             tc.tile_pool(name="ins", bufs=1) as ins, \
             tc.tile_pool(name="persist", bufs=1) as persist, \
             tc.tile_pool(name="work", bufs=2) as work, \
             tc.tile_pool(name="es", bufs=16) as esp, \
             tc.tile_pool(name="avn", bufs=2) as avn, \
             tc.tile_pool(name="po", bufs=2) as pop, \
             tc.tile_pool(name="ps", bufs=3, space="PSUM") as ps, \
             tc.tile_pool(name="avps", bufs=1, space="PSUM") as avps:

            # ---- loads ----
            bdm = consts.tile([128, 128], F32R)
            bds = consts.tile([128, 128], F32R)
            pbt = consts.tile([128, 3], F32)
            ddof_b = consts.tile([128, 1], F32)
            nc.vector.memset(ddof_b[:], -0.5 * float(np.log(64.0 / 63.0)))
            nc.sync.dma_start(out=bdm[:], in_=bd_mean[:, :].bitcast(F32R))
            nc.sync.dma_start(out=bds[:], in_=bd_ssq[:, :].bitcast(F32R))
            nc.sync.dma_start(out=pbt[:], in_=pb[:, :])

            xt = ins.tile([128, KC, N], F32R)
            wq = ins.tile([128, KC, 3 * C], F32R)
            wp = ins.tile([128, KC, C], F32R)
            # HAM warmup: keep TensorE busy during the input DMA wait so the
            # clock gate opens (cold matmuls run at 1.2GHz, warm at 2.4GHz)
            warm_ps = ps.tile([64, 512], F32, tag="big")
            warm_w = bdm[:].bitcast(BF16)
            warm_rhs = bass.AP(
                tensor=warm_w.tensor, offset=warm_w.offset,
                ap=[list(warm_w.ap[0]), [0, 4], [1, 128]])  # [128, 4, 128] step-0
            for _ in range(24):
                nc.tensor.matmul(warm_ps[:], warm_w[:, 0:64], warm_rhs,
                                 start=True, stop=True)

            xr = xT.rearrange("(k p) n -> p k n", p=128).bitcast(F32R)
            wr = qkv_wT.rearrange("(k p) m -> p k m", p=128).bitcast(F32R)
            for k in range(KC):
                nc.sync.dma_start(out=wq[:, k, 0:128], in_=wr[:, k, 0:128])
                nc.scalar.dma_start(out=xt[:, k, :], in_=xr[:, k, :])
            for k in range(KC):
                nc.sync.dma_start(out=wq[:, k, 384:512], in_=wr[:, k, 384:512])
            for k in range(KC):
                nc.sync.dma_start(out=wq[:, k, 128:384], in_=wr[:, k, 128:384])
                nc.sync.dma_start(out=wq[:, k, 512:1152], in_=wr[:, k, 512:1152])
            nc.sync.dma_start(
                out=wp[:], in_=proj_wT.rearrange("(k p) m -> p k m", p=128).bitcast(F32R))

            vo = persist.tile([128, TC, H, 128], BF16)
            nc.vector.memset(vo[:, :, :, D:128], 1.0)

            # ---- phase emitters (software-pipelined across head pairs) ----
            qn = persist.tile([128, 2 * H, N], BF16)    # normalized q|k (bf16 -> FWL)
            aoT = persist.tile([128, KC, N], F32R)
            scale = float(D) ** -0.5

            def emit_qkv_norm(j):
                """QKV chunk j (128-row mode) + normalization over d.
                Processed per 512-column half to halve the chain latency
                that gates each pair's QK^T start."""
                qk_ps = ps.tile([128, N], F32, tag="big")
                for k in range(KC):
                    for h5 in range(HALF):
                        nc.tensor.matmul(
                            qk_ps[:, h5 * 512:(h5 + 1) * 512],
                            wq[:, k, j * 128:(j + 1) * 128],
                            xt[:, k, h5 * 512:(h5 + 1) * 512],
                            start=(k == 0), stop=(k == KC - 1))
                for h5 in range(HALF):
                    sl = slice(h5 * 512, (h5 + 1) * 512)
                    qkr = work.tile([128, 512], F32R, tag="qkr")
                    nc.vector.tensor_copy(qkr[:], qk_ps[:, sl])
                    mean_ps = ps.tile([128, 512], F32, tag="big")
                    nc.tensor.matmul(mean_ps[:], bdm[:], qkr[:],
                                     start=True, stop=True)
                    qc = work.tile([128, 512], F32R, tag="qc")
                    nc.vector.tensor_sub(qc[:], qkr[:], mean_ps[:])
                    qc2 = work.tile([128, 512], F32R, tag="qc2")
                    nc.vector.tensor_mul(qc2[:], qc[:], qc[:])
                    ssq_ps = ps.tile([128, 512], F32, tag="big")
                    nc.tensor.matmul(ssq_ps[:], bds[:], qc2[:],
                                     start=True, stop=True)
                    # rstd = exp(-0.5*ln(ssq/64) + ddof bias) — Rsqrt/Reciprocal
                    # ACT tables are blocked; ln/exp share the softmax table set
                    lnv = work.tile([128, 512], F32R, tag="lnv")
                    nc.scalar.activation(lnv[:], ssq_ps[:], LN)
                    rstd = work.tile([128, 512], F32R, tag="rstd")
                    nc.scalar.activation(rstd[:], lnv[:], EXP, scale=-0.5,
                                         bias=ddof_b[:])
                    nc.vector.tensor_mul(qn[:, j, sl], qc[:], rstd[:])

            def emit_v():
                """V token-major (128-row mode), ones cols for denominators."""
                for t in range(TC):
                    v_ps = ps.tile([128, C], F32, tag="big")
                    for k in range(KC):
                        nc.tensor.matmul(
                            v_ps[:],
                            xt[:, k, t * 128:(t + 1) * 128],
                            wq[:, k, 2 * C:3 * C],
                            start=(k == 0), stop=(k == KC - 1))
                    nc.vector.tensor_copy(
                        vo[:, t, :, 0:D],
                        v_ps[:].rearrange("p (h d) -> p h d", h=H))

            def emit_pair(j):
                """Attention for heads 2j, 2j+1. QK^T in 64-row PE mode with
                the heads on row groups T0/T8, strictly alternating so weight
                loads overlap the other group's matmuls; AV afterwards in
                128-row mode (grouped with the next pair's QKV/norm to
                minimize PE mode-switch drains)."""
                es_tiles = {0: [], 1: []}
                for t in range(TC):
                    s_e = ps.tile([128, N], F32, tag="big")
                    s_o = ps.tile([128, N], F32, tag="big")
                    s_tiles = {0: s_e, 1: s_o}
                    for h5 in range(HALF):
                        for p in range(2):
                            lo = p * 64
                            nc.tensor.matmul(
                                s_tiles[p][:, h5 * 512:(h5 + 1) * 512],
                                qn[lo:lo + 64, 3 + j, t * 128:(t + 1) * 128],
                                qn[lo:lo + 64, j, h5 * 512:(h5 + 1) * 512],
                                start=True, stop=True)
                    for p in range(2):
                        es = esp.tile([128, N], BF16, tag="es")
                        nc.scalar.activation(es[:], s_tiles[p][:], EXP, scale=scale)
                        es_tiles[p].append(es)
                for p in range(2):
                    av_ps = avps.tile([128, N], F32, tag="av")
                    for t in range(TC):
                        for h5 in range(HALF):
                            nc.tensor.matmul(
                                av_ps[:, h5 * 512:(h5 + 1) * 512],
                                vo[:, t, 2 * j + p, :],
                                es_tiles[p][t][:, h5 * 512:(h5 + 1) * 512],
                                start=(t == 0), stop=(t == TC - 1))
                    # rows 64-127 hold the softmax denominator already
                    # broadcast over 64 partitions; 1/d = exp(-ln d) on ACT
                    lnd = avn.tile([64, N], F32R, tag="lnd")
                    nc.scalar.activation(lnd[:], av_ps[D:128, :], LN)
                    rec_b = avn.tile([64, N], F32R, tag="rec_b")
                    nc.scalar.activation(rec_b[:], lnd[:], EXP, scale=-1.0)
                    lo = p * 64
                    nc.vector.tensor_mul(aoT[lo:lo + 64, j, :], av_ps[0:D, :], rec_b[:])

            # pipeline: pair j's ACT-heavy phase overlaps pair j+1's PE-heavy QKV
            emit_qkv_norm(0)
            emit_qkv_norm(3)
            emit_v()
            emit_pair(0)
            emit_qkv_norm(1)
            emit_qkv_norm(4)
            emit_pair(1)
            emit_qkv_norm(2)
            emit_qkv_norm(5)
            emit_pair(2)

            # ---- output projection ----
            for co in range(KC):
                p_ps = ps.tile([128, N], F32, tag="big")
                for h5 in range(HALF):
                    for k in range(KC):
                        nc.tensor.matmul(
                            p_ps[:, h5 * 512:(h5 + 1) * 512],
                            wp[:, k, co * 128:(co + 1) * 128],
                            aoT[:, k, h5 * 512:(h5 + 1) * 512],
                            start=(k == 0), stop=(k == KC - 1))
                po = pop.tile([128, N], F32, tag="po")
                nc.vector.tensor_scalar_add(po[:], p_ps[:], pbt[:, co:co + 1])
                nc.sync.dma_start(out=outT[co * 128:(co + 1) * 128, :], in_=po[:])

    return nc


def _get_prog():
    global _prog
    if _prog is None:
        _prog = _build()
    return _prog


def _make_in_maps(x, qkv_w, proj_w, proj_b):
    qkv_wT = np.ascontiguousarray(np.asarray(qkv_w, np.float32).T)
    proj_wT = np.ascontiguousarray(np.asarray(proj_w, np.float32).T)
    pb = np.ascontiguousarray(
        np.asarray(proj_b, np.float32).reshape(3, 128).T)
    bd_mean = np.zeros((128, 128), np.float32)
    bd_ssq = np.zeros((128, 128), np.float32)
    for b0 in (0, 64):
        bd_mean[b0:b0 + 64, b0:b0 + 64] = 1.0 / D
        bd_ssq[b0:b0 + 64, b0:b0 + 64] = 1.0 / D   # ddof fix in rstd exp bias

    shared = {
        "qkv_wT": qkv_wT, "proj_wT": proj_wT, "pb": pb,
        "bd_mean": bd_mean, "bd_ssq": bd_ssq,
    }
    x = np.asarray(x, np.float32)
    return [
        {"xT": np.ascontiguousarray(x[b].T), **shared} for b in range(B)
    ]


def run(x, qkv_w, proj_w, proj_b, trace=False):
    from concourse.bass_utils import run_bass_kernel_spmd

    nc = _get_prog()
    in_maps = _make_in_maps(x, qkv_w, proj_w, proj_b)
    res = run_bass_kernel_spmd(
        nc, in_maps, core_ids=list(range(NCORES)), trace=trace)
    out = np.stack([res.results[b]["outT"].T for b in range(B)])
    return np.ascontiguousarray(out.astype(np.float32)), res


def kernel(x, qkv_w, proj_w, proj_b):
    out, _ = run(x, qkv_w, proj_w, proj_b)
    return out



# revision 3
# speedup vs baseline: 1.0575x; 1.0575x over previous
"""Trainium2 Bass kernel for nn_Attention_41085657153633.

B=8, N=1024, C=384, H=6, D=64 attention with per-head q/k normalization
(mean/std over head_dim, ddof=1), softmax(QK^T/sqrt(D)) @ V, output proj.

Sharding: data-parallel over B — each of the 8 NeuronCores handles one
batch element end-to-end (no collectives).

Per-core dataflow (fp32r matmuls; bf16 for the exp-score/V stage):
  - QKV^T computed head-major for Q,K ([d on partitions, tokens free] —
    the layout QK^T wants) and token-major for V. V stored as
    [v_h | 64 ones cols] per head so AV's rows 64-127 accumulate the
    softmax denominator broadcast across 64 partitions for free.
  - q/k normalization over the partition (d) axis: mean via
    blockdiag(1/64) matmul; variance PACKED compactly — two accumulating
    matmuls with ones-column stationaries land sum(qc^2)/63 for the q-
    and k-chunk of a head pair into one [4,512] PSUM tile, so the ACT
    ln/exp rsqrt runs once per (pair, half) instead of over a full
    128-partition broadcast; a tiny [4->128] ones matmul broadcasts the
    rstd back for the DVE normalize multiply.
  - scores S^T[kt,qt] per head via K^T x Q^T; head pairs occupy PE row
    groups T0/T8 (64-row mode) so the pair's matmuls run concurrently.
  - exp on ACT with 1/sqrt(D) folded in (max-sub skipped: |S/8| <~ 7);
    bf16 scores feed AV. AV runs in qt-halves ([128,512] PSUM tiles) to
    keep the PSUM bank budget at 8: 2x score double-buffer (4) +
    2 AV accumulators (2) + 2 rotating norm banks (2).
  - softmax denominator reciprocal via ACT exp(-ln d); ln/exp share one
    table set with the softmax exp — no table switches.
  - emission interleaves pair j's AV-half-0 into the QK/exp stream, then
    pair j+1's norm, then AV-half-1, so TensorE never idles long enough
    for the HAM clock gate to re-throttle; a warmup matmul burst opens
    the gate during the input DMA wait.
  - output proj from attn_out^T with the bias add on ACT (idle at tail);
    host transposes.
"""

import sys

sys.path.insert(0, "/opt/trn_rl_repo")

import json

import numpy as np

B, N, C = 8, 1024, 384
H, D = 6, 64
NCORES = 8

_prog = None


def _install_multiwait_fixup():
    """This container's walrus build rejects >1 sync wait per instruction
    ("Too many sync wait commands"). Rewrite the BIR JSON before compile:
    hoist extra waits onto single-wait EventSemaphore instructions
    inserted just before the owner on the same engine (engines dispatch
    in program order, so the gating is preserved)."""
    from concourse import bass2jax, bass_utils

    if getattr(bass_utils, "_multiwait_fixup", False):
        return
    bass_utils._multiwait_fixup = True

    orig = bass_utils.compile_bir_kernel

    def _split(bir_json: bytes) -> bytes:
        j = json.loads(bir_json)
        for fn in j.get("functions", []):
            for bb in fn.get("blocks", []):
                out = []
                for inst in bb.get("instructions", []):
                    si = inst.get("sync_info")
                    waits = si.get("on_wait", []) if si else []
                    if len(waits) > 1:
                        for k, w in enumerate(waits[:-1]):
                            out.append({
                                "debug": inst.get("debug", 0),
                                "engine": inst["engine"],
                                "ins": [],
                                "outs": [],
                                "name": f"{inst['name']}-sw{k}",
                                "opcode": "EventSemaphore",
                                "sync_info": {"on_update": [], "on_wait": [w]},
                            })
                        si["on_wait"] = [waits[-1]]
                    out.append(inst)
                bb["instructions"] = out
        return json.dumps(j).encode()

    def patched(bir_json, tmpdir, neff_name="file.neff"):
        return orig(_split(bir_json), tmpdir, neff_name)

    bass_utils.compile_bir_kernel = patched
    bass2jax.compile_bir_kernel = patched


def _build():
    import concourse.bass as bass
    import concourse.tile as tile
    from concourse import mybir

    _install_multiwait_fixup()

    F32 = mybir.dt.float32
    F32R = mybir.dt.float32r
    BF16 = mybir.dt.bfloat16
    EXP = mybir.ActivationFunctionType.Exp
    LN = mybir.ActivationFunctionType.Ln
    IDENT = mybir.ActivationFunctionType.Identity

    nc = bass.Bass("TRN2")
    xT = nc.dram_tensor("xT", [C, N], F32, kind="ExternalInput")
    qkv_wT = nc.dram_tensor("qkv_wT", [C, 3 * C], F32, kind="ExternalInput")
    proj_wT = nc.dram_tensor("proj_wT", [C, C], F32, kind="ExternalInput")
    pb = nc.dram_tensor("pb", [128, 3], F32, kind="ExternalInput")
    bd_mean = nc.dram_tensor("bd_mean", [128, 128], F32, kind="ExternalInput")
    ssq_sel = nc.dram_tensor("ssq_sel", [128, 8], F32, kind="ExternalInput")
    bc_sel = nc.dram_tensor("bc_sel", [4, 256], F32, kind="ExternalInput")
    outT = nc.dram_tensor("outT", [C, N], F32, kind="ExternalOutput")

    KC = C // 128   # 3 contraction chunks of the model dim
    TC = N // 128   # 8 token chunks
    scale = float(D) ** -0.5

    with tile.TileContext(nc) as tc:
      with nc.allow_low_precision(reason="f32r/bf16 matmul intermediates"):
        with tc.tile_pool(name="consts", bufs=1) as consts, \
             tc.tile_pool(name="ins", bufs=1) as ins, \
             tc.tile_pool(name="persist", bufs=1) as persist, \
             tc.tile_pool(name="work", bufs=2) as work, \
             tc.tile_pool(name="qcp", bufs=3) as qcp, \
             tc.tile_pool(name="small", bufs=2) as small, \
             tc.tile_pool(name="es", bufs=16) as esp, \
             tc.tile_pool(name="avn", bufs=2) as avn, \
             tc.tile_pool(name="po", bufs=2) as pop, \
             tc.tile_pool(name="scp", bufs=2, space="PSUM") as scp, \
             tc.tile_pool(name="avp", bufs=2, space="PSUM") as avp, \
             tc.tile_pool(name="psA", bufs=2, space="PSUM") as psA:

            # ---- const + input loads ----
            bdm = consts.tile([128, 128], F32R)
            sels = consts.tile([128, 8], F32R)
            bsel = consts.tile([4, 256], F32R)
            pbt = consts.tile([128, 3], F32)
            nc.sync.dma_start(out=bdm[:], in_=bd_mean[:, :].bitcast(F32R))
            nc.sync.dma_start(out=sels[:], in_=ssq_sel[:, :].bitcast(F32R))
            nc.sync.dma_start(out=bsel[:], in_=bc_sel[:, :].bitcast(F32R))
            nc.sync.dma_start(out=pbt[:], in_=pb[:, :])

            xt = ins.tile([128, KC, N], F32R)
            wq = ins.tile([128, KC, 3 * C], F32R)
            wp = ins.tile([128, KC, C], F32R)

            # HAM warmup: keep TensorE busy during the input DMA wait so the
            # clock gate opens (cold matmuls run at 1.2GHz, warm at 2.4GHz)
            warm_ps = scp.tile([64, 512], F32, tag="s")
            warm_w = bdm[:].bitcast(BF16)
            warm_rhs = bass.AP(
                tensor=warm_w.tensor, offset=warm_w.offset,
                ap=[list(warm_w.ap[0]), [0, 4], [1, 128]])  # [128, 4, 128] step-0
            for _ in range(28):
                nc.tensor.matmul(warm_ps[:], warm_w[:, 0:64], warm_rhs,
                                 start=True, stop=True)

            xr = xT.rearrange("(k p) n -> p k n", p=128).bitcast(F32R)
            wr = qkv_wT.rearrange("(k p) m -> p k m", p=128).bitcast(F32R)
            for k in range(KC):
                nc.sync.dma_start(out=wq[:, k, 0:128], in_=wr[:, k, 0:128])
                nc.scalar.dma_start(out=xt[:, k, :], in_=xr[:, k, :])
            for k in range(KC):
                nc.sync.dma_start(out=wq[:, k, 384:512], in_=wr[:, k, 384:512])
            for k in range(KC):
                nc.sync.dma_start(out=wq[:, k, 768:1152], in_=wr[:, k, 768:1152])
            for k in range(KC):
                nc.sync.dma_start(out=wq[:, k, 128:256], in_=wr[:, k, 128:256])
                nc.sync.dma_start(out=wq[:, k, 512:640], in_=wr[:, k, 512:640])
            for k in range(KC):
                nc.sync.dma_start(out=wq[:, k, 256:384], in_=wr[:, k, 256:384])
                nc.sync.dma_start(out=wq[:, k, 640:768], in_=wr[:, k, 640:768])
            nc.sync.dma_start(
                out=wp[:], in_=proj_wT.rearrange("(k p) m -> p k m", p=128).bitcast(F32R))

            vo = persist.tile([128, TC, H, 128], BF16)
            nc.vector.memset(vo[:, :, :, D:128], 1.0)

            qn = persist.tile([128, 2 * H, N], BF16)    # normalized q|k
            aoT = persist.tile([128, KC, N], F32R)

            # ---- norm for one head pair: chunks j (q) and 3+j (k) ----
            def emit_norm(j):
                for h in range(2):
                    sl = slice(h * 512, (h + 1) * 512)
                    qcs = {}
                    for ci, c in enumerate((j, 3 + j)):
                        qk_ps = psA.tile([128, 512], F32, tag="psA")
                        for k in range(KC):
                            nc.tensor.matmul(
                                qk_ps[:],
                                wq[:, k, c * 128:(c + 1) * 128],
                                xt[:, k, sl],
                                start=(k == 0), stop=(k == KC - 1))
                        qkr = work.tile([128, 512], F32R, tag="qkr")
                        nc.vector.tensor_copy(qkr[:], qk_ps[:])
                        mean_ps = psA.tile([128, 512], F32, tag="psA")
                        nc.tensor.matmul(mean_ps[:], bdm[:], qkr[:],
                                         start=True, stop=True)
                        qc = qcp.tile([128, 512], F32R, tag="qc")
                        nc.vector.tensor_sub(qc[:], qkr[:], mean_ps[:])
                        qc2 = work.tile([128, 512], F32R, tag="qc2")
                        nc.vector.tensor_mul(qc2[:], qc[:], qc[:])
                        qcs[c] = (qc, qc2)
                    # packed variance: q-chunk -> rows 0,1; k-chunk -> rows
                    # 2,3 of one [4,512] PSUM tile (has_written accumulate)
                    pack = psA.tile([4, 512], F32, tag="psA")
                    nc.tensor.matmul(pack[:], sels[:, 0:4], qcs[j][1][:],
                                     start=True, stop=False)
                    nc.tensor.matmul(pack[:], sels[:, 4:8], qcs[3 + j][1][:],
                                     start=False, stop=True)
                    # rstd = exp(-0.5 ln(var)); ln/exp share the softmax set
                    lnp = small.tile([4, 512], F32R, tag="lnp")
                    nc.scalar.activation(lnp[:], pack[:], LN)
                    rsp = small.tile([4, 512], F32R, tag="rsp")
                    nc.scalar.activation(rsp[:], lnp[:], EXP, scale=-0.5)
                    for c, bs in ((j, bsel[:, 0:128]), (3 + j, bsel[:, 128:256])):
                        rb = psA.tile([128, 512], F32, tag="psA")
                        nc.tensor.matmul(rb[:], bs, rsp[:],
                                         start=True, stop=True)
                        nc.vector.tensor_mul(qn[:, c, sl], qcs[c][0][:], rb[:])

            def emit_v():
                """V token-major (128-row mode), ones cols for denominators."""
                for t in range(TC):
                    v_ps = psA.tile([128, C], F32, tag="psA")
                    for k in range(KC):
                        nc.tensor.matmul(
                            v_ps[:],
                            xt[:, k, t * 128:(t + 1) * 128],
                            wq[:, k, 2 * C:3 * C],
                            start=(k == 0), stop=(k == KC - 1))
                    nc.vector.tensor_copy(
                        vo[:, t, :, 0:D],
                        v_ps[:].rearrange("p (h d) -> p h d", h=H))

            def emit_denom(j, av_tiles, h):
                sl = slice(h * 512, (h + 1) * 512)
                for p in range(2):
                    lnd = avn.tile([64, 512], F32R, tag="lnd")
                    nc.scalar.activation(lnd[:], av_tiles[p][64:128, :], LN)
                    rec = avn.tile([64, 512], F32R, tag="rec")
                    nc.scalar.activation(rec[:], lnd[:], EXP, scale=-1.0)
                    lo = p * 64
                    nc.vector.tensor_mul(aoT[lo:lo + 64, j, sl],
                                         av_tiles[p][0:D, :], rec[:])

            def emit_pair(j):
                """Attention for heads 2j, 2j+1. QK^T in 64-row PE mode with
                heads on row groups T0/T8 (concurrent); AV accumulates in
                qt-halves with half-0 interleaved into the QK/exp stream."""
                av0 = {p: avp.tile([128, 512], F32, tag="av", name=f"av0_{p}") for p in range(2)}
                es_all = []
                for t in range(TC):
                    s = {p: scp.tile([128, N], F32, tag="s", name=f"s_{p}") for p in range(2)}
                    for h5 in range(2):
                        for p in range(2):
                            lo = p * 64
                            nc.tensor.matmul(
                                s[p][:, h5 * 512:(h5 + 1) * 512],
                                qn[lo:lo + 64, 3 + j, t * 128:(t + 1) * 128],
                                qn[lo:lo + 64, j, h5 * 512:(h5 + 1) * 512],
                                start=True, stop=True)
                    es_t = {}
                    for p in range(2):
                        es = esp.tile([128, N], BF16, tag="es")
                        nc.scalar.activation(es[:], s[p][:], EXP, scale=scale)
                        es_t[p] = es
                        nc.tensor.matmul(av0[p][:],
                                         vo[:, t, 2 * j + p, :],
                                         es[:, 0:512],
                                         start=(t == 0), stop=(t == TC - 1))
                    es_all.append(es_t)
                emit_denom(j, av0, 0)
                if j < 2:
                    emit_norm(j + 1)
                av1 = {p: avp.tile([128, 512], F32, tag="av", name=f"av1_{p}") for p in range(2)}
                for t in range(TC):
                    for p in range(2):
                        nc.tensor.matmul(av1[p][:],
                                         vo[:, t, 2 * j + p, :],
                                         es_all[t][p][:, 512:1024],
                                         start=(t == 0), stop=(t == TC - 1))
                emit_denom(j, av1, 1)

            # ---- pipeline ----
            emit_norm(0)
            emit_v()
            emit_pair(0)
            emit_pair(1)
            emit_pair(2)

            # ---- output projection (bias add on ACT — idle at tail) ----
            for co in range(KC):
                p_ps = scp.tile([128, N], F32, tag="s")
                for h5 in range(2):
                    for k in range(KC):
                        nc.tensor.matmul(
                            p_ps[:, h5 * 512:(h5 + 1) * 512],
                            wp[:, k, co * 128:(co + 1) * 128],
                            aoT[:, k, h5 * 512:(h5 + 1) * 512],
                            start=(k == 0), stop=(k == KC - 1))
                po = pop.tile([128, N], F32, tag="po")
                nc.scalar.activation(po[:], p_ps[:], IDENT,
                                     bias=pbt[:, co:co + 1])
                nc.sync.dma_start(out=outT[co * 128:(co + 1) * 128, :], in_=po[:])

    return nc


def _get_prog():
    global _prog
    if _prog is None:
        _prog = _build()
    return _prog


def _make_in_maps(x, qkv_w, proj_w, proj_b):
    qkv_wT = np.ascontiguousarray(np.asarray(qkv_w, np.float32).T)
    proj_wT = np.ascontiguousarray(np.asarray(proj_w, np.float32).T)
    pb = np.ascontiguousarray(
        np.asarray(proj_b, np.float32).reshape(3, 128).T)
    bd_mean = np.zeros((128, 128), np.float32)
    for b0 in (0, 64):
        bd_mean[b0:b0 + 64, b0:b0 + 64] = 1.0 / D
    # packed-variance stationaries: cols 0-3 for the q-chunk (rows 0,1 of
    # the pack), cols 4-7 for the k-chunk (rows 2,3); 1/63 = ddof=1
    ssq_sel = np.zeros((128, 8), np.float32)
    ssq_sel[0:64, 0] = 1.0 / 63.0
    ssq_sel[64:128, 1] = 1.0 / 63.0
    ssq_sel[0:64, 6] = 1.0 / 63.0
    ssq_sel[64:128, 7] = 1.0 / 63.0
    # broadcast-back: [4 -> 128] ones; cols 0:128 read pack rows 0,1 (q),
    # cols 128:256 read pack rows 2,3 (k)
    bc_sel = np.zeros((4, 256), np.float32)
    bc_sel[0, 0:64] = 1.0
    bc_sel[1, 64:128] = 1.0
    bc_sel[2, 128 + 0:128 + 64] = 1.0
    bc_sel[3, 128 + 64:128 + 128] = 1.0

    shared = {
        "qkv_wT": qkv_wT, "proj_wT": proj_wT, "pb": pb,
        "bd_mean": bd_mean, "ssq_sel": ssq_sel, "bc_sel": bc_sel,
    }
    x = np.asarray(x, np.float32)
    return [
        {"xT": np.ascontiguousarray(x[b].T), **shared} for b in range(B)
    ]


def run(x, qkv_w, proj_w, proj_b, trace=False):
    from concourse.bass_utils import run_bass_kernel_spmd

    nc = _get_prog()
    in_maps = _make_in_maps(x, qkv_w, proj_w, proj_b)
    res = run_bass_kernel_spmd(
        nc, in_maps, core_ids=list(range(NCORES)), trace=trace)
    out = np.stack([res.results[b]["outT"].T for b in range(B)])
    return np.ascontiguousarray(out.astype(np.float32)), res


def kernel(x, qkv_w, proj_w, proj_b):
    out, _ = run(x, qkv_w, proj_w, proj_b)
    return out


# revision 5
# speedup vs baseline: 1.1567x; 1.0938x over previous
"""Trainium2 Bass kernel for nn_Attention_41085657153633.

B=8, N=1024, C=384, H=6, D=64 attention with per-head q/k normalization
(mean/std over head_dim, ddof=1), softmax(QK^T/sqrt(D)) @ V, output proj.

Sharding: data-parallel over B — each of the 8 NeuronCores handles one
batch element end-to-end (no collectives).

Per-core dataflow (fp32r matmuls; bf16 for the exp-score/V stage):
  - QKV^T computed head-major for Q,K ([d on partitions, tokens free] —
    the layout QK^T wants) and token-major for V. V stored as
    [v_h | 64 ones cols] per head so AV's rows 64-127 accumulate the
    softmax denominator broadcast across 64 partitions for free.
  - q/k normalization over the partition (d) axis: mean via
    blockdiag(1/64) matmul; variance PACKED compactly — two accumulating
    matmuls with ones-column stationaries land sum(qc^2)/63 for the q-
    and k-chunk of a head pair into one [4,512] PSUM tile, so the ACT
    ln/exp rsqrt runs once per (pair, half) instead of over a full
    128-partition broadcast; a tiny [4->128] ones matmul broadcasts the
    rstd back for the DVE normalize multiply.
  - scores S^T[kt,qt] per head via K^T x Q^T; head pairs occupy PE row
    groups T0/T8 (64-row mode) so the pair's matmuls run concurrently.
  - exp on ACT with 1/sqrt(D) folded in (max-sub skipped: |S/8| <~ 7);
    bf16 scores feed AV. AV runs in qt-halves ([128,512] PSUM tiles) to
    keep the PSUM bank budget at 8: 2x score double-buffer (4) +
    2 AV accumulators (2) + 2 rotating norm banks (2).
  - softmax denominator reciprocal via ACT exp(-ln d); ln/exp share one
    table set with the softmax exp — no table switches.
  - per-t V matmuls (pair 0) and the next pair's norm half-units are
    emitted INSIDE the QK/exp t-loop so TensorE has dense fill work
    during the ACT-paced exp stream (HAM clock-gate warmth) and the next
    pair's rstd is ready before its QK starts (no ACT gap at pair
    boundaries); AV-half-1 runs as a dense sprint after the loop.
  - x DMA split across 3 queues and issued first — it gates everything.
  - output proj from attn_out^T with the bias add on ACT (idle at tail);
    host transposes.
"""

import sys

sys.path.insert(0, "/opt/trn_rl_repo")

import json

import numpy as np

B, N, C = 8, 1024, 384
H, D = 6, 64
NCORES = 8

_prog = None


def _install_multiwait_fixup():
    """This container's walrus build rejects >1 sync wait per instruction
    ("Too many sync wait commands"). Rewrite the BIR JSON before compile:
    hoist extra waits onto single-wait EventSemaphore instructions
    inserted just before the owner on the same engine (engines dispatch
    in program order, so the gating is preserved)."""
    from concourse import bass2jax, bass_utils

    if getattr(bass_utils, "_multiwait_fixup", False):
        return
    bass_utils._multiwait_fixup = True

    orig = bass_utils.compile_bir_kernel

    def _split(bir_json: bytes) -> bytes:
        j = json.loads(bir_json)
        for fn in j.get("functions", []):
            for bb in fn.get("blocks", []):
                out = []
                for inst in bb.get("instructions", []):
                    si = inst.get("sync_info")
                    waits = si.get("on_wait", []) if si else []
                    if len(waits) > 1:
                        for k, w in enumerate(waits[:-1]):
                            out.append({
                                "debug": inst.get("debug", 0),
                                "engine": inst["engine"],
                                "ins": [],
                                "outs": [],
                                "name": f"{inst['name']}-sw{k}",
                                "opcode": "EventSemaphore",
                                "sync_info": {"on_update": [], "on_wait": [w]},
                            })
                        si["on_wait"] = [waits[-1]]
                    out.append(inst)
                bb["instructions"] = out
        return json.dumps(j).encode()

    def patched(bir_json, tmpdir, neff_name="file.neff"):
        return orig(_split(bir_json), tmpdir, neff_name)

    bass_utils.compile_bir_kernel = patched
    bass2jax.compile_bir_kernel = patched


def _build():
    import concourse.bass as bass
    import concourse.tile as tile
    from concourse import mybir

    _install_multiwait_fixup()

    F32 = mybir.dt.float32
    F32R = mybir.dt.float32r
    BF16 = mybir.dt.bfloat16
    EXP = mybir.ActivationFunctionType.Exp
    LN = mybir.ActivationFunctionType.Ln
    IDENT = mybir.ActivationFunctionType.Identity

    nc = bass.Bass("TRN2")
    xT = nc.dram_tensor("xT", [C, N], F32, kind="ExternalInput")
    qkv_wT = nc.dram_tensor("qkv_wT", [C, 3 * C], F32, kind="ExternalInput")
    proj_wT = nc.dram_tensor("proj_wT", [C, C], F32, kind="ExternalInput")
    pb = nc.dram_tensor("pb", [128, 3], F32, kind="ExternalInput")
    bd_mean = nc.dram_tensor("bd_mean", [128, 128], F32, kind="ExternalInput")
    ssq_sel = nc.dram_tensor("ssq_sel", [128, 8], F32, kind="ExternalInput")
    bc_sel = nc.dram_tensor("bc_sel", [4, 256], F32, kind="ExternalInput")
    outT = nc.dram_tensor("outT", [C, N], F32, kind="ExternalOutput")

    KC = C // 128   # 3 contraction chunks of the model dim
    TC = N // 128   # 8 token chunks
    scale = float(D) ** -0.5

    with tile.TileContext(nc) as tc:
      with nc.allow_low_precision(reason="f32r/bf16 matmul intermediates"):
        with tc.tile_pool(name="consts", bufs=1) as consts, \
             tc.tile_pool(name="ins", bufs=1) as ins, \
             tc.tile_pool(name="persist", bufs=1) as persist, \
             tc.tile_pool(name="work", bufs=2) as work, \
             tc.tile_pool(name="qcp", bufs=3) as qcp, \
             tc.tile_pool(name="small", bufs=2) as small, \
             tc.tile_pool(name="es", bufs=16) as esp, \
             tc.tile_pool(name="avn", bufs=2) as avn, \
             tc.tile_pool(name="po", bufs=2) as pop, \
             tc.tile_pool(name="scp", bufs=2, space="PSUM") as scp, \
             tc.tile_pool(name="avp", bufs=2, space="PSUM") as avp, \
             tc.tile_pool(name="psA", bufs=2, space="PSUM") as psA:

            xt = ins.tile([128, KC, N], F32R)
            wq = ins.tile([128, KC, 3 * C], F32R)
            wp = ins.tile([128, KC, C], F32R)

            # ---- x first (it gates everything), split across 3 queues ----
            xr = xT.rearrange("(k p) n -> p k n", p=128).bitcast(F32R)
            wr = qkv_wT.rearrange("(k p) m -> p k m", p=128).bitcast(F32R)
            nc.scalar.dma_start(out=xt[:, 0, :], in_=xr[:, 0, :])
            nc.gpsimd.dma_start(out=xt[:, 1, :], in_=xr[:, 1, :])
            nc.scalar.dma_start(out=xt[:, 2, :], in_=xr[:, 2, :])

            bdm = consts.tile([128, 128], F32R)
            sels = consts.tile([128, 8], F32R)
            bsel = consts.tile([4, 256], F32R)
            pbt = consts.tile([128, 3], F32)
            nc.sync.dma_start(out=bdm[:], in_=bd_mean[:, :].bitcast(F32R))
            nc.sync.dma_start(out=sels[:], in_=ssq_sel[:, :].bitcast(F32R))
            nc.sync.dma_start(out=bsel[:], in_=bc_sel[:, :].bitcast(F32R))
            nc.sync.dma_start(out=pbt[:], in_=pb[:, :])

            # HAM warmup: keep TensorE busy during the input DMA wait so the
            # clock gate opens (cold matmuls run at 1.2GHz, warm at 2.4GHz)
            warm_ps = scp.tile([64, 512], F32, tag="s")
            warm_w = bdm[:].bitcast(BF16)
            warm_rhs = bass.AP(
                tensor=warm_w.tensor, offset=warm_w.offset,
                ap=[list(warm_w.ap[0]), [0, 4], [1, 128]])  # [128, 4, 128] step-0
            for _ in range(28):
                nc.tensor.matmul(warm_ps[:], warm_w[:, 0:64], warm_rhs,
                                 start=True, stop=True)

            # weights: pair-0 chunks first, then V, then pairs 1/2
            for k in range(KC):
                nc.sync.dma_start(out=wq[:, k, 0:128], in_=wr[:, k, 0:128])
                nc.sync.dma_start(out=wq[:, k, 384:512], in_=wr[:, k, 384:512])
            for k in range(KC):
                nc.sync.dma_start(out=wq[:, k, 768:1152], in_=wr[:, k, 768:1152])
            for k in range(KC):
                nc.sync.dma_start(out=wq[:, k, 128:256], in_=wr[:, k, 128:256])
                nc.sync.dma_start(out=wq[:, k, 512:640], in_=wr[:, k, 512:640])
            for k in range(KC):
                nc.sync.dma_start(out=wq[:, k, 256:384], in_=wr[:, k, 256:384])
                nc.sync.dma_start(out=wq[:, k, 640:768], in_=wr[:, k, 640:768])
            nc.sync.dma_start(
                out=wp[:], in_=proj_wT.rearrange("(k p) m -> p k m", p=128).bitcast(F32R))

            vo = persist.tile([128, TC, H, 128], BF16)
            nc.vector.memset(vo[:, :, :, D:128], 1.0)

            qn = persist.tile([128, 2 * H, N], BF16)    # normalized q|k
            aoT = persist.tile([128, KC, N], F32R)

            def emit_norm_half(j, h):
                """Norm for chunks j (q) and 3+j (k), token half h."""
                sl = slice(h * 512, (h + 1) * 512)
                qcs = {}
                for c in (j, 3 + j):
                    qk_ps = psA.tile([128, 512], F32, tag="psA", name="qk_ps")
                    for k in range(KC):
                        nc.tensor.matmul(
                            qk_ps[:],
                            wq[:, k, c * 128:(c + 1) * 128],
                            xt[:, k, sl],
                            start=(k == 0), stop=(k == KC - 1))
                    qkr = work.tile([128, 512], F32R, tag="qkr", name="qkr")
                    nc.vector.tensor_copy(qkr[:], qk_ps[:])
                    mean_ps = psA.tile([128, 512], F32, tag="psA", name="mean_ps")
                    nc.tensor.matmul(mean_ps[:], bdm[:], qkr[:],
                                     start=True, stop=True)
                    qc = qcp.tile([128, 512], F32R, tag="qc", name="qc")
                    nc.vector.tensor_sub(qc[:], qkr[:], mean_ps[:])
                    qc2 = work.tile([128, 512], F32R, tag="qc2", name="qc2")
                    nc.vector.tensor_mul(qc2[:], qc[:], qc[:])
                    qcs[c] = (qc, qc2)
                # packed variance: q-chunk -> rows 0,1; k-chunk -> rows 2,3
                # of one [4,512] PSUM tile (has_written accumulate)
                pack = psA.tile([4, 512], F32, tag="psA", name="pack")
                nc.tensor.matmul(pack[:], sels[:, 0:4], qcs[j][1][:],
                                 start=True, stop=False)
                nc.tensor.matmul(pack[:], sels[:, 4:8], qcs[3 + j][1][:],
                                 start=False, stop=True)
                # rstd = exp(-0.5 ln(var)); ln/exp share the softmax set
                lnp = small.tile([4, 512], F32R, tag="lnp", name="lnp")
                nc.scalar.activation(lnp[:], pack[:], LN)
                rsp = small.tile([4, 512], F32R, tag="rsp", name="rsp")
                nc.scalar.activation(rsp[:], lnp[:], EXP, scale=-0.5)
                for c, bs in ((j, bsel[:, 0:128]), (3 + j, bsel[:, 128:256])):
                    rb = psA.tile([128, 512], F32, tag="psA", name="rb")
                    nc.tensor.matmul(rb[:], bs, rsp[:], start=True, stop=True)
                    nc.vector.tensor_mul(qn[:, c, sl], qcs[c][0][:], rb[:])

            def emit_v_t(t):
                """V token-major for one token chunk, ones cols pre-set."""
                v_ps = psA.tile([128, C], F32, tag="psA", name="v_ps")
                for k in range(KC):
                    nc.tensor.matmul(
                        v_ps[:],
                        xt[:, k, t * 128:(t + 1) * 128],
                        wq[:, k, 2 * C:3 * C],
                        start=(k == 0), stop=(k == KC - 1))
                nc.vector.tensor_copy(
                    vo[:, t, :, 0:D],
                    v_ps[:].rearrange("p (h d) -> p h d", h=H))

            def emit_denom(j, av_tiles, h):
                sl = slice(h * 512, (h + 1) * 512)
                for p in range(2):
                    lnd = avn.tile([64, 512], F32R, tag="lnd", name="lnd")
                    nc.scalar.activation(lnd[:], av_tiles[p][64:128, :], LN)
                    rec = avn.tile([64, 512], F32R, tag="rec", name="rec")
                    nc.scalar.activation(rec[:], lnd[:], EXP, scale=-1.0)
                    lo = p * 64
                    nc.vector.tensor_mul(aoT[lo:lo + 64, j, sl],
                                         av_tiles[p][0:D, :], rec[:])

            def emit_pair(j):
                """Attention for heads 2j, 2j+1. QK^T in 64-row PE mode with
                heads on row groups T0/T8 (concurrent); AV accumulates in
                qt-halves with half-0 interleaved into the QK/exp stream.
                Pair 0 also computes V per token chunk in-loop; the next
                pair's norm half-units are emitted mid-loop as PE fill."""
                av0 = {p: avp.tile([128, 512], F32, tag="av", name=f"av0_{p}")
                       for p in range(2)}
                es_all = []
                for t in range(TC):
                    if j == 0:
                        emit_v_t(t)
                    s = {p: scp.tile([128, N], F32, tag="s", name=f"s_{p}")
                         for p in range(2)}
                    for h5 in range(2):
                        for p in range(2):
                            lo = p * 64
                            nc.tensor.matmul(
                                s[p][:, h5 * 512:(h5 + 1) * 512],
                                qn[lo:lo + 64, 3 + j, t * 128:(t + 1) * 128],
                                qn[lo:lo + 64, j, h5 * 512:(h5 + 1) * 512],
                                start=True, stop=True)
                    es_t = {}
                    for p in range(2):
                        es = esp.tile([128, N], BF16, tag="es", name="es")
                        nc.scalar.activation(es[:], s[p][:], EXP, scale=scale)
                        es_t[p] = es
                        nc.tensor.matmul(av0[p][:],
                                         vo[:, t, 2 * j + p, :],
                                         es[:, 0:512],
                                         start=(t == 0), stop=(t == TC - 1))
                    es_all.append(es_t)
                    if j < 2 and t == 2:
                        emit_norm_half(j + 1, 0)
                    if j < 2 and t == 5:
                        emit_norm_half(j + 1, 1)
                emit_denom(j, av0, 0)
                av1 = {p: avp.tile([128, 512], F32, tag="av", name=f"av1_{p}")
                       for p in range(2)}
                for t in range(TC):
                    for p in range(2):
                        nc.tensor.matmul(av1[p][:],
                                         vo[:, t, 2 * j + p, :],
                                         es_all[t][p][:, 512:1024],
                                         start=(t == 0), stop=(t == TC - 1))
                emit_denom(j, av1, 1)

            # ---- pipeline ----
            emit_norm_half(0, 0)
            emit_norm_half(0, 1)
            emit_pair(0)
            emit_pair(1)
            emit_pair(2)

            # ---- output projection (bias add on ACT — idle at tail) ----
            for co in range(KC):
                p_ps = scp.tile([128, N], F32, tag="s", name="p_ps")
                for h5 in range(2):
                    for k in range(KC):
                        nc.tensor.matmul(
                            p_ps[:, h5 * 512:(h5 + 1) * 512],
                            wp[:, k, co * 128:(co + 1) * 128],
                            aoT[:, k, h5 * 512:(h5 + 1) * 512],
                            start=(k == 0), stop=(k == KC - 1))
                po = pop.tile([128, N], F32, tag="po", name="po")
                nc.scalar.activation(po[:], p_ps[:], IDENT,
                                     bias=pbt[:, co:co + 1])
                nc.sync.dma_start(out=outT[co * 128:(co + 1) * 128, :], in_=po[:])

    return nc


def _get_prog():
    global _prog
    if _prog is None:
        _prog = _build()
    return _prog


def _make_in_maps(x, qkv_w, proj_w, proj_b):
    qkv_wT = np.ascontiguousarray(np.asarray(qkv_w, np.float32).T)
    proj_wT = np.ascontiguousarray(np.asarray(proj_w, np.float32).T)
    pb = np.ascontiguousarray(
        np.asarray(proj_b, np.float32).reshape(3, 128).T)
    bd_mean = np.zeros((128, 128), np.float32)
    for b0 in (0, 64):
        bd_mean[b0:b0 + 64, b0:b0 + 64] = 1.0 / D
    # packed-variance stationaries: cols 0-3 for the q-chunk (rows 0,1 of
    # the pack), cols 4-7 for the k-chunk (rows 2,3); 1/63 = ddof=1
    ssq_sel = np.zeros((128, 8), np.float32)
    ssq_sel[0:64, 0] = 1.0 / 63.0
    ssq_sel[64:128, 1] = 1.0 / 63.0
    ssq_sel[0:64, 6] = 1.0 / 63.0
    ssq_sel[64:128, 7] = 1.0 / 63.0
    # broadcast-back: [4 -> 128] ones; cols 0:128 read pack rows 0,1 (q),
    # cols 128:256 read pack rows 2,3 (k)
    bc_sel = np.zeros((4, 256), np.float32)
    bc_sel[0, 0:64] = 1.0
    bc_sel[1, 64:128] = 1.0
    bc_sel[2, 128 + 0:128 + 64] = 1.0
    bc_sel[3, 128 + 64:128 + 128] = 1.0

    shared = {
        "qkv_wT": qkv_wT, "proj_wT": proj_wT, "pb": pb,
        "bd_mean": bd_mean, "ssq_sel": ssq_sel, "bc_sel": bc_sel,
    }
    x = np.asarray(x, np.float32)
    return [
        {"xT": np.ascontiguousarray(x[b].T), **shared} for b in range(B)
    ]


def run(x, qkv_w, proj_w, proj_b, trace=False):
    from concourse.bass_utils import run_bass_kernel_spmd

    nc = _get_prog()
    in_maps = _make_in_maps(x, qkv_w, proj_w, proj_b)
    res = run_bass_kernel_spmd(
        nc, in_maps, core_ids=list(range(NCORES)), trace=trace)
    out = np.stack([res.results[b]["outT"].T for b in range(B)])
    return np.ascontiguousarray(out.astype(np.float32)), res


def kernel(x, qkv_w, proj_w, proj_b):
    out, _ = run(x, qkv_w, proj_w, proj_b)
    return out
